# revision 8
# baseline (speedup 1.0000x reference)
"""Dense transformer layer (B2 S2048 D1024 H16) on 8 Trainium2 cores.

Data-parallel over rows: core c handles batch c//4, query rows (c%4)*512.
Each core redundantly computes K/V for its batch element (no collectives).

v2d: bf16 matmul operands; KT SBUF-resident; attnV stationaries padded to
128 columns (M<128 stationaries serialize LDWEIGHTS on HW: 440 vs 165
ns/MM); head pairs stacked on partitions 0-63/64-127 so out_proj runs
K=128; reciprocal_approx_fast instead of reciprocal (3.6us -> 0.7us each).

v_sb block layout per (chunk, pair): [V_even(64) | ones+pad(64) | V_odd(64)]
 - even stationary = cols 0:128   -> psum rows 0-63 attn, row 64 denom
 - odd  stationary = cols 64:192  -> psum row 0 denom, rows 64-127 attn
"""
from contextlib import ExitStack

import concourse.bass as bass
import concourse.mybir as mybir
import concourse.tile as tile
from concourse import bacc

F32 = mybir.dt.float32
F32R = mybir.dt.float32r
BF16 = mybir.dt.bfloat16
AF = mybir.ActivationFunctionType
ALU = mybir.AluOpType
LN_EPS = 1e-5


def build(S=2048, SQ=512, D=1024, H=16, DFF=4096, n_cores=8,
          compile=True, pt_bufs=6, sc_bufs=2, wp_bufs=8, wfp_bufs=8,
          drain_bufs=3, fdr_bufs=3, sq_bufs=3, wop_bufs=8, vps_bufs=8,
          fps_bufs=8, reps=1, skip=(), **_unused):
    P = 128
    HD = D // H                      # 64
    HP = H // 2                      # 8 head pairs
    DC = D // P                      # 8 feature chunks
    FC = DFF // P                    # 32 ffn chunks
    SC = S // P                      # 16 key-row chunks
    NQ = SQ
    assert NQ <= 512
    SB = min(512, S)
    VB = 192                         # v_sb pair-block width

    cast = lambda ap: ap

    nc = bacc.Bacc("TRN2", target_bir_lowering=False, num_devices=n_cores)

    xT = nc.dram_tensor("xT", [D, S], BF16, kind="ExternalInput")
    xqT = nc.dram_tensor("xqT", [D, SQ], BF16, kind="ExternalInput")
    W = {}
    for name, shp in [("Wq", [D, D]), ("Wk", [D, D]), ("Wv", [D, D]),
                      ("Wo", [D, D]), ("W1", [D, DFF]), ("W2", [DFF, D])]:
        W[name] = nc.dram_tensor(name, shp, BF16, kind="ExternalInput")
    vecs = {}
    for name, n in [("bqs", D), ("bk", D), ("bv", D), ("bo", D), ("bf1", DFF),
                    ("bf2", D), ("g1", D), ("b1n", D), ("g2", D), ("b2n", D)]:
        vecs[name] = nc.dram_tensor(name, [n], F32, kind="ExternalInput")
    ones_d = nc.dram_tensor("ones_d", [S // P * H], BF16, kind="ExternalInput")
    yT = nc.dram_tensor("yT", [D, SQ], F32, kind="ExternalOutput")

    with tile.TileContext(nc) as tc, ExitStack() as top:
        const = top.enter_context(tc.tile_pool(name="const", bufs=1))

        vt = {}
        for name, n in [("bqs", D), ("bk", D), ("bo", D), ("bf1", DFF),
                        ("bf2", D), ("g1", D), ("b1n", D), ("g2", D),
                        ("b2n", D)]:
            t = const.tile([P, n // P], F32, tag=f"vec_{name}")
            nc.sync.dma_start(out=t, in_=vecs[name].ap().rearrange(
                "(t p) -> p t", p=P))
            vt[name] = t
        bvb = const.tile([P, D], F32, tag="bvb")  # bv broadcast over rows
        nc.sync.dma_start(out=bvb, in_=vecs["bv"].ap().partition_broadcast(P))
        ones_sq = const.tile([P, P], BF16, tag="ones_sq")   # LN sum stationary
        nc.sync.dma_start(out=ones_sq,
                          in_=ones_d.ap()[0:P].partition_broadcast(P))
        ones_row = const.tile([1, P], BF16, tag="ones_row")  # bcast stationary
        nc.sync.dma_start(out=ones_row, in_=ones_d.ap()[0:P])
        eps_t = const.tile([1, 1], F32, tag="eps_t")
        nc.vector.memset(eps_t, LN_EPS)

        for _rep in range(reps):
            # pools that cross phase boundaries (midp before actp: LIFO order)
            midp_cm = tc.tile_pool(name="midp", bufs=1)
            midp = midp_cm.__enter__()
            ln_in = midp.tile([P, DC, NQ], BF16, tag="ln_in")
            actp_cm = tc.tile_pool(name="actp", bufs=1)
            actp = actp_cm.__enter__()
            qt = actp.tile([P, DC, NQ], BF16, tag="qt")    # QT (Wq pre-scaled)
            kt_sb = actp.tile([P, DC, S], BF16, tag="kt_sb")  # KT resident
            xq = actp.tile([P, DC, NQ], BF16, tag="xq")
            v_sb = actp.tile([P, SC, HP, VB], BF16, tag="v_sb")

            # ================= phase 1: projections =================
            with ExitStack() as ph:
                p1 = ph.enter_context(tc.tile_pool(name="p1", bufs=1))
                wp = ph.enter_context(tc.tile_pool(name="wp", bufs=wp_bufs))
                psum = ph.enter_context(tc.tile_pool(name="ps1", bufs=vps_bufs,
                                                     space="PSUM"))

                # ---- QT first: only needs xq, so PE starts early ----
                for c in range(DC):
                    nc.sync.dma_start(out=xq[:, c, :],
                                      in_=xqT[c * P:(c + 1) * P, :])
                pss = [psum.tile([P, NQ], F32, tag="pp", name=f"pp_q{i}")
                       for i in range(DC)]
                for c in range(DC):
                    wt = wp.tile([P, D], BF16, tag="wt", name=f"wqt{c}")
                    nc.sync.dma_start(out=wt, in_=W["Wq"][c * P:(c + 1) * P, :])
                    for t in range(DC):
                        nc.tensor.matmul(
                            pss[t], cast(wt[:, t * P:(t + 1) * P]),
                            cast(xq[:, c, :]),
                            start=(c == 0), stop=(c == DC - 1))
                for t in range(DC):
                    nc.vector.tensor_scalar(
                        out=qt[:, t, :], in0=pss[t],
                        scalar1=vt["bqs"][:, t:t + 1], scalar2=None,
                        op0=ALU.add)

                SH = S // 2                      # rows per half
                RBH = SH // SB                   # 512-blocks per half (2)
                SCH = SH // P                    # 128-chunks per half (8)
                if "kv" in skip:
                    nc.vector.memset(kt_sb, 0.0)
                    nc.vector.memset(v_sb, 1.0)
                else:
                    # ones for denominators + junk pad (cols 64-127 of blocks)
                    nc.vector.memset(v_sb[:, :, :, HD:2 * HD], 1.0)
                for half in (() if "kv" in skip else range(2)):
                    xtc = [p1.tile([P, SH], BF16, tag=f"xt{c}",
                                   name=f"xt{half}_{c}") for c in range(DC)]
                    for c in range(DC):
                        nc.sync.dma_start(
                            out=xtc[c],
                            in_=xT[c * P:(c + 1) * P, half * SH:(half + 1) * SH])

                    # ---- KT = Wk.T @ x.T -> kt_sb (SBUF resident) ----
                    for th in range(2):
                        pss = [psum.tile([P, SB], F32, tag="pp",
                                         name=f"pp_kt{half}_{th}_{i}")
                               for i in range(8)]
                        for c in range(DC):
                            wt = wp.tile([P, 4 * P], BF16, tag="wt",
                                         name=f"wkt{half}_{th}_{c}")
                            nc.sync.dma_start(
                                out=wt,
                                in_=W["Wk"][c * P:(c + 1) * P,
                                            th * 4 * P:(th + 1) * 4 * P])
                            for t4 in range(4):
                                for rb in range(RBH):
                                    nc.tensor.matmul(
                                        pss[t4 * RBH + rb],
                                        cast(wt[:, t4 * P:(t4 + 1) * P]),
                                        cast(xtc[c][:, rb * SB:(rb + 1) * SB]),
                                        start=(c == 0), stop=(c == DC - 1))
                        for t4 in range(4):
                            t = th * 4 + t4
                            for rb in range(RBH):
                                nc.vector.tensor_scalar(
                                    out=kt_sb[:, t,
                                              half * SH + rb * SB:
                                              half * SH + (rb + 1) * SB],
                                    in0=pss[t4 * RBH + rb],
                                    scalar1=vt["bk"][:, t:t + 1], scalar2=None,
                                    op0=ALU.add)

                    # ---- V natural [s, dout] -> v_sb pair blocks ----
                    DB = 512
                    for dv in range(D // DB):
                        pss = [psum.tile([P, DB], F32, tag="pp",
                                         name=f"pp_v{half}_{dv}_{i}")
                               for i in range(8)]
                        for c in range(DC):
                            wt = wp.tile([P, DB], BF16, tag="wt",
                                         name=f"wvt{half}_{dv}_{c}")
                            nc.sync.dma_start(
                                out=wt,
                                in_=W["Wv"][c * P:(c + 1) * P,
                                            dv * DB:(dv + 1) * DB])
                            for r8 in range(SCH):
                                nc.tensor.matmul(
                                    pss[r8],
                                    cast(xtc[c][:, r8 * P:(r8 + 1) * P]),
                                    cast(wt),
                                    start=(c == 0), stop=(c == DC - 1))
                        for r8 in range(SCH):
                            rt = half * SCH + r8
                            pv = pss[r8].rearrange("p (hp e d) -> p hp e d",
                                                   e=2, d=HD)
                            bv4 = bvb[:, dv * DB:(dv + 1) * DB].rearrange(
                                "p (hp e d) -> p hp e d", e=2, d=HD)
                            for e in range(2):
                                nc.vector.tensor_add(
                                    v_sb[:, rt, dv * 4:(dv + 1) * 4,
                                         e * 2 * HD:e * 2 * HD + HD],
                                    pv[:, :, e, :], bv4[:, :, e, :])

            # ================= phase 2: attention =================
            attnp_cm = tc.tile_pool(name="attnp", bufs=1)
            attnp = attnp_cm.__enter__()
            attn_pair = attnp.tile([P, HP, NQ], BF16, tag="attn_pair")
            with ExitStack() as ph:
                pt_p = ph.enter_context(tc.tile_pool(name="pt", bufs=pt_bufs))
                nrm = ph.enter_context(tc.tile_pool(name="nrm", bufs=4))
                sc_ps = ph.enter_context(tc.tile_pool(name="scp", bufs=sc_bufs,
                                                      space="PSUM"))
                at_ps = ph.enter_context(tc.tile_pool(name="atp", bufs=2,
                                                      space="PSUM"))
                bc_ps = ph.enter_context(tc.tile_pool(name="bcp", bufs=2,
                                                      space="PSUM"))
                for hp in range(HP):
                    kth = kt_sb[:, hp, :]
                    aps = [at_ps.tile([P, NQ], F32, tag="aps",
                                      name=f"aps{hp}_{j}") for j in range(2)]
                    # head-packed per-chunk tiles: one [P, 2(heads), NQ] psum
                    # per chunk so sc_bufs=2 keeps TWO chunks in flight and
                    # the ACT exp stream never waits on scores
                    for ck in range(SC):
                        sps = sc_ps.tile([P, 2, NQ], F32, tag="sp",
                                         name=f"sp{hp}_{ck}")
                        for hh, pol in ((0, 0), (1, HD)):
                            nc.tensor.matmul(
                                sps[:, hh, :],
                                cast(kth[pol:pol + HD, ck * P:(ck + 1) * P]),
                                cast(qt[pol:pol + HD, hp, :]),
                                start=True, stop=True)
                        pt = pt_p.tile([P, 2, NQ], BF16, tag="pt",
                                       name=f"pt{hp}_{ck}")
                        nc.scalar.activation(out=pt, in_=sps, func=AF.Exp)
                        for hh in range(2):
                            nc.tensor.matmul(
                                aps[hh],
                                cast(v_sb[:, ck, hp, hh * HD:hh * HD + P]),
                                cast(pt[:, hh, :]),
                                start=(ck == 0), stop=(ck == SC - 1))
                    # normalize: even denom at aps[0] row 64, odd at aps[1] row 0
                    rcp_e = nrm.tile([1, NQ], F32, tag="rcpe",
                                     name=f"rcpe{hp}")
                    rcp_o = nrm.tile([1, NQ], F32, tag="rcpo",
                                     name=f"rcpo{hp}")
                    den_e = nrm.tile([1, NQ], F32, tag="dene",
                                     name=f"dene{hp}")
                    # custom DVE op reads partition 0 only -> move row 64 down
                    nc.vector.tensor_copy(out=den_e, in_=aps[0][HD:HD + 1, :])
                    nc.vector.reciprocal_approx_fast(out=rcp_e, in_=den_e)
                    nc.vector.reciprocal_approx_fast(out=rcp_o,
                                                     in_=aps[1][0:1, :])
                    rcp_eb = nrm.tile([1, NQ], BF16, tag="rcpeb",
                                      name=f"rcpeb{hp}")
                    rcp_ob = nrm.tile([1, NQ], BF16, tag="rcpob",
                                      name=f"rcpob{hp}")
                    with nc.allow_low_precision(reason="bf16 bcast feed"):
                        nc.vector.tensor_copy(out=rcp_eb, in_=rcp_e)
                        nc.vector.tensor_copy(out=rcp_ob, in_=rcp_o)
                    bcp = bc_ps.tile([P, NQ], F32, tag="bcp", name=f"bcp{hp}")
                    nc.tensor.matmul(bcp[0:HD, :], cast(ones_row[:, 0:HD]),
                                     cast(rcp_eb), start=True, stop=True)
                    nc.tensor.matmul(bcp[HD:P, :], cast(ones_row[:, 0:HD]),
                                     cast(rcp_ob), start=True, stop=True,
                                     tile_position=(0, HD))
                    bcs = nrm.tile([P, NQ], F32, tag="bcs", name=f"bcs{hp}")
                    nc.vector.tensor_copy(out=bcs, in_=bcp)
                    with nc.allow_low_precision(reason="bf16 attn"):
                        nc.vector.tensor_mul(attn_pair[0:HD, hp, :],
                                             aps[0][0:HD, :], bcs[0:HD, :])
                        nc.vector.tensor_mul(attn_pair[HD:P, hp, :],
                                             aps[1][HD:P, :], bcs[HD:P, :])

            # ---- out-proj (K=128 per head pair) + residual -> ln_in ----
            with ExitStack() as ph:
                wop = ph.enter_context(tc.tile_pool(name="wop", bufs=wop_bufs))
                odr = ph.enter_context(tc.tile_pool(name="odr", bufs=3))
                op_ps = ph.enter_context(tc.tile_pool(name="opp", bufs=8,
                                                      space="PSUM"))
                wts_o = []
                for hp in range(HP):
                    wt = wop.tile([P, D], BF16, tag="wot", name=f"wot{hp}")
                    nc.sync.dma_start(out=wt, in_=W["Wo"][hp * P:(hp + 1) * P, :])
                    wts_o.append(wt)
                # two 4-bank passes so LN1's stats PSUM can coexist with the
                # second pass (LN1 sums start as soon as ln_in chunks land)
                for g in range(2):
                    pss = [op_ps.tile([P, NQ], F32, tag="op",
                                      name=f"op_{g}_{i}") for i in range(4)]
                    for hp in range(HP):
                        for t4 in range(4):
                            t = g * 4 + t4
                            nc.tensor.matmul(
                                pss[t4], cast(wts_o[hp][:, t * P:(t + 1) * P]),
                                cast(attn_pair[:, hp, :]),
                                start=(hp == 0), stop=(hp == HP - 1))
                    for t4 in range(4):
                        t = g * 4 + t4
                        oa = odr.tile([P, NQ], F32, tag="oa")
                        nc.vector.tensor_scalar(
                            out=oa, in0=pss[t4], scalar1=vt["bo"][:, t:t + 1],
                            scalar2=None, op0=ALU.add)
                        with nc.allow_low_precision(reason="bf16 resid"):
                            nc.vector.tensor_add(ln_in[:, t, :], oa,
                                                 xq[:, t, :])

            attnp_cm.__exit__(None, None, None)  # free attn_pair
            actp_cm.__exit__(None, None, None)   # free qt, kt_sb, xq, v_sb

            lnp_cm = tc.tile_pool(name="lnp", bufs=1)
            lnp = lnp_cm.__enter__()

            def layer_norm(ph, src, g, b, tag, out_dtype=BF16):
                """src [P, DC, NQ] feature-major -> LN'd tile from lnp."""
                st_ps = ph.enter_context(tc.tile_pool(name=f"st{tag}", bufs=1,
                                                      space="PSUM"))
                bb_ps = ph.enter_context(tc.tile_pool(name=f"bb{tag}", bufs=2,
                                                      space="PSUM"))
                sqp = ph.enter_context(tc.tile_pool(name=f"sq{tag}", bufs=sq_bufs))
                row = ph.enter_context(tc.tile_pool(name=f"row{tag}", bufs=1))

                # sums via M=128 all-ones stationary (M<128 is slow on HW);
                # row 0 of the psum holds the result
                sm = st_ps.tile([P, NQ], F32, tag="sm")
                sq = st_ps.tile([P, NQ], F32, tag="sq")
                for c in range(DC):
                    nc.tensor.matmul(sm, cast(ones_sq), cast(src[:, c, :]),
                                     start=(c == 0), stop=(c == DC - 1))
                for c in range(DC):
                    x2 = sqp.tile([P, NQ], BF16, tag="x2")
                    nc.scalar.activation(out=x2, in_=src[:, c, :], func=AF.Square)
                    nc.tensor.matmul(sq, cast(ones_sq), cast(x2),
                                     start=(c == 0), stop=(c == DC - 1))
                mean = row.tile([1, NQ], F32, tag="mean")
                nc.scalar.mul(out=mean, in_=sm[0:1, :], mul=1.0 / D)
                msq = row.tile([1, NQ], F32, tag="msq")
                nc.scalar.mul(out=msq, in_=sq[0:1, :], mul=1.0 / D)
                var = row.tile([1, NQ], F32, tag="var")
                nc.vector.tensor_mul(var, mean, mean)
                nc.vector.tensor_sub(var, msq, var)
                sd = row.tile([1, NQ], F32, tag="sd")
                nc.scalar.activation(out=sd, in_=var, func=AF.Sqrt, bias=eps_t)
                rstd = row.tile([1, NQ], F32, tag="rstd")
                nc.vector.reciprocal_approx_fast(out=rstd, in_=sd)
                rstd_b = row.tile([1, NQ], BF16, tag="rstd_b")
                nc.scalar.mul(out=rstd_b, in_=rstd, mul=1.0)
                shift = row.tile([1, NQ], F32, tag="shift")   # -mean*rstd
                nc.vector.tensor_mul(shift, mean, rstd)
                shift_b = row.tile([1, NQ], BF16, tag="shift_b")
                nc.scalar.mul(out=shift_b, in_=shift, mul=-1.0)

                ab = bb_ps.tile([P, NQ], F32, tag="ab")
                nc.tensor.matmul(ab, cast(ones_row), cast(rstd_b),
                                 start=True, stop=True)
                a_b = sqp.tile([P, NQ], F32, tag="a_b")
                nc.vector.tensor_copy(out=a_b, in_=ab)
                bb = bb_ps.tile([P, NQ], F32, tag="ab")
                nc.tensor.matmul(bb, cast(ones_row), cast(shift_b),
                                 start=True, stop=True)
                b_b = sqp.tile([P, NQ], F32, tag="b_b")
                nc.vector.tensor_copy(out=b_b, in_=bb)

                out_t = lnp.tile([P, DC, NQ], out_dtype, tag=f"ln{tag}")
                for c in range(DC):
                    tmp = sqp.tile([P, NQ], F32, tag="tmp")
                    nc.vector.tensor_mul(tmp, src[:, c, :], a_b)
                    nc.vector.tensor_add(tmp, tmp, b_b)
                    nc.scalar.activation(
                        out=out_t[:, c, :], in_=tmp, func=AF.Identity,
                        scale=g[:, c:c + 1], bias=b[:, c:c + 1])
                return out_t

            # ================= phase 3: LN1, FFN, LN2 =================
            with ExitStack() as ph:
                ln1 = layer_norm(ph, ln_in, vt["g1"], vt["b1n"], "1")

            with ExitStack() as ph:
                ffn = ph.enter_context(tc.tile_pool(name="ffn", bufs=1))
                wfp = ph.enter_context(tc.tile_pool(name="wfp", bufs=wfp_bufs))
                fdr = ph.enter_context(tc.tile_pool(name="fdr", bufs=fdr_bufs))
                f_ps = ph.enter_context(tc.tile_pool(name="fps", bufs=fps_bufs,
                                                     space="PSUM"))
                h1 = ffn.tile([P, FC, NQ], BF16, tag="h1")
                for pg in (() if "ffn" in skip else range(FC // 8)):
                    pss = [f_ps.tile([P, NQ], F32, tag="fp",
                                     name=f"fp1_{pg}_{i}") for i in range(8)]
                    for c in range(DC):
                        wt = wfp.tile([P, 8 * P], BF16, tag="w1t")
                        nc.sync.dma_start(
                            out=wt,
                            in_=W["W1"][c * P:(c + 1) * P,
                                        pg * 8 * P:(pg + 1) * 8 * P])
                        for t8 in range(8):
                            nc.tensor.matmul(
                                pss[t8], cast(wt[:, t8 * P:(t8 + 1) * P]),
                                cast(ln1[:, c, :]),
                                start=(c == 0), stop=(c == DC - 1))
                    for t8 in range(8):
                        t = pg * 8 + t8
                        nc.scalar.activation(
                            out=h1[:, t, :], in_=pss[t8], func=AF.Gelu,
                            bias=vt["bf1"][:, t:t + 1])

                ln2_in = midp.tile([P, DC, NQ], BF16, tag="ln2_in")
                if "ffn" in skip:
                    for t in range(DC):
                        nc.vector.tensor_copy(out=ln2_in[:, t, :],
                                              in_=ln1[:, t, :])
                else:
                    pss = [f_ps.tile([P, NQ], F32, tag="fp", name=f"fp2_{i}")
                           for i in range(DC)]
                    for c in range(FC):
                        wt = wfp.tile([P, D], BF16, tag="w2t")
                        nc.sync.dma_start(out=wt,
                                          in_=W["W2"][c * P:(c + 1) * P, :])
                        for t in range(DC):
                            nc.tensor.matmul(
                                pss[t], cast(wt[:, t * P:(t + 1) * P]),
                                cast(h1[:, c, :]),
                                start=(c == 0), stop=(c == FC - 1))
                    for t in range(DC):
                        fo = fdr.tile([P, NQ], F32, tag="fo")
                        nc.vector.tensor_scalar(
                            out=fo, in0=pss[t], scalar1=vt["bf2"][:, t:t + 1],
                            scalar2=None, op0=ALU.add)
                        with nc.allow_low_precision(reason="bf16 resid"):
                            nc.vector.tensor_add(ln2_in[:, t, :], fo,
                                                 ln1[:, t, :])

            with ExitStack() as ph:
                y_out = layer_norm(ph, ln2_in, vt["g2"], vt["b2n"], "2",
                                   out_dtype=F32)
                yv = yT.ap().rearrange("(t p) q -> p t q", p=P)
                for c in range(DC):
                    nc.sync.dma_start(out=yv[:, c, :], in_=y_out[:, c, :])
            lnp_cm.__exit__(None, None, None)
            midp_cm.__exit__(None, None, None)

    if compile:
        nc.compile()
    return nc


# ---------------- host-side sharding / gather ----------------
import numpy as np
import ml_dtypes

B, S, D, H = 2, 2048, 1024, 16
HD = D // H
DFF = 4 * D
N_CORES = 8
CPB = N_CORES // B           # cores per batch element
SQ = S // CPB                # query rows per core

_nc = None


def _get_nc():
    global _nc
    if _nc is None:
        _nc = build(S=S, SQ=SQ, D=D, H=H, DFF=DFF, n_cores=N_CORES)
    return _nc


def _make_in_maps(inputs):
    bf16 = ml_dtypes.bfloat16
    x = np.ascontiguousarray(inputs["x"], dtype=np.float32)
    scale = np.float32(1.0 / np.sqrt(HD))
    shared = {
        "Wq": (np.asarray(inputs["Wq"], np.float32) * scale).astype(bf16),
        "Wk": np.asarray(inputs["Wk"], np.float32).astype(bf16),
        "Wv": np.asarray(inputs["Wv"], np.float32).astype(bf16),
        "Wo": np.asarray(inputs["Wo"], np.float32).astype(bf16),
        "W1": np.asarray(inputs["W1"], np.float32).astype(bf16),
        "W2": np.asarray(inputs["W2"], np.float32).astype(bf16),
        "bqs": np.ascontiguousarray(inputs["bq"], np.float32) * scale,
        "bk": np.ascontiguousarray(inputs["bk"], np.float32),
        "bv": np.ascontiguousarray(inputs["bv"], np.float32),
        "bo": np.ascontiguousarray(inputs["bo"], np.float32),
        "bf1": np.ascontiguousarray(inputs["bf1"], np.float32),
        "bf2": np.ascontiguousarray(inputs["bf2"], np.float32),
        "g1": np.ascontiguousarray(inputs["g1"], np.float32),
        "b1n": np.ascontiguousarray(inputs["b1n"], np.float32),
        "g2": np.ascontiguousarray(inputs["g2"], np.float32),
        "b2n": np.ascontiguousarray(inputs["b2n"], np.float32),
        "ones_d": np.ones((S // 128) * H, bf16),
    }
    xT = np.ascontiguousarray(x.transpose(0, 2, 1)).astype(bf16)  # [B, D, S]
    in_maps = []
    for c in range(N_CORES):
        b, q0 = c // CPB, (c % CPB) * SQ
        m = dict(shared)
        m["xT"] = xT[b]
        m["xqT"] = np.ascontiguousarray(xT[b][:, q0:q0 + SQ])
        in_maps.append(m)
    return in_maps


def kernel(**inputs):
    from concourse.bass_utils import run_bass_kernel_spmd
    nc = _get_nc()
    in_maps = _make_in_maps(inputs)
    res = run_bass_kernel_spmd(nc, in_maps, core_ids=list(range(N_CORES)))
    y = np.empty((B, S, D), dtype=np.float32)
    for c in range(N_CORES):
        b, q0 = c // CPB, (c % CPB) * SQ
        y[b, q0:q0 + SQ, :] = res.results[c]["yT"].T
    return y
